# revision 16
# baseline (speedup 1.0000x reference)
"""Trainium2 Bass kernel for nn_AttMatch (2-graph attention + SAGEConv GNN).

Self-contained: takes the full unsharded inputs of the reference problem,
shards across 8 NeuronCores internally, runs one SPMD NEFF, and gathers the
full [8192, 8192] sigmoid adjacency output.

Implementation notes:
  * layer phases ordered for comms overlap: kv-proj -> q(0) -> attn(0) ->
    q(1) -> attn(1) -> xsage(0) -> tail(0) -> xsage(1) -> tail(1); q(1)
    emitted after attn(0) so the PE never head-of-line blocks on the
    previous layer's last AllGather.
  * one uniform [128,512] PSUM tag rotating over 4 banks: rotation depth 4
    breaks the exp->scores serialization chain that stalls the PE.
  * exp(scores) at [128,512] granularity: DVE Schraudolph (uint8 bit trick
    -> fp8e4m3) for query-half 0, scalar exact Exp->fp8 for half 1.
  * alpha^T@v and softmax column sums as fp8 DoubleRow matmuls (K=256 per
    instruction) accumulating in PSUM; M^T resident in SBUF as fp8 and both
    M-aggregations fp8 DoubleRow.
  * AllReduce per 2048-query half (bf16), launched mid-attention; a dummy
    warm-up AllReduce at kernel start absorbs cross-core skew.
  * final sigmoid(F@F^T) computes a balanced circulant 9/16 of the
    symmetric output (host mirrors the rest); the two own-column units
    (k=0,8) read local features and overlap the last AllGather; sigmoid
    splits between scalar and a DVE Schraudolph chain. Output bf16.
"""

import numpy as np
import ml_dtypes

import concourse.bass as bass
import concourse.bacc as bacc
import concourse.tile as tile
import concourse.mybir as mybir
from concourse.bass_utils import run_bass_kernel_spmd

BF16 = ml_dtypes.bfloat16
E4M3 = ml_dtypes.float8_e4m3

N = 4096          # nodes per graph
D = 128           # feature dim
NCORES = 8
SH = N // NCORES  # 512 nodes per graph per core
ICW = 1024        # query-chunk width
NIC = N // ICW    # 4 query chunks
NT = 2 * SH // 128   # 8 local target tiles
NTP = NT // 2     # 4 target-tile pairs (DoubleRow)
NJ = N // 128     # 32 source-node tiles
NJP = NJ // 2     # 16 source-tile pairs
KU = 9            # circulant column units in the final phase
INV_SCALE = 1.0 / np.sqrt(128.0)

V_SCALE = 0.25         # v pre-scale (headroom in bf16/fp8 paths)
CS_SCALE = 1.0 / 64.0  # ones value for column sums
REP_SCALE = V_SCALE / CS_SCALE  # =16: makes prod = P/colsum exactly
AR_FP8 = False  # AllReduce payload dtype (fp8 halves CC time; bf16 fallback)

LOG2E = 1.4426950408889634
ES_A = INV_SCALE * 8.0 * LOG2E
ES_B = 56.0 - 0.344 + 0.5
SIG_A = float(1 << 23) * LOG2E
SIG_B = 127.0 * float(1 << 23) - 360768.0
SIG_LO = -1.0e9
SIG_HI = 2.09e9

F32 = mybir.dt.float32
BF = mybir.dt.bfloat16
FP8 = mybir.dt.float8e4
U8 = mybir.dt.uint8
I32 = mybir.dt.int32
DR = mybir.MatmulPerfMode.DoubleRow

ADD = mybir.AluOpType.add
SUB = mybir.AluOpType.subtract
MULT = mybir.AluOpType.mult
MAX = mybir.AluOpType.max
MIN = mybir.AluOpType.min
EXP = mybir.ActivationFunctionType.Exp
IDN = mybir.ActivationFunctionType.Identity
CPY = mybir.ActivationFunctionType.Copy
SIG = mybir.ActivationFunctionType.Sigmoid

WK, WQ, WV, WLS, WL1, WRS, WR1N = range(7)
IDENT = 14
BK, BQ, BV, BL = range(4)

_cache = {}


def _build_nc():
    nc = bacc.Bacc("TRN2", target_bir_lowering=False, debug=False,
                   num_devices=NCORES)

    # ---- external I/O ----
    x1t = nc.dram_tensor("x1t", [D, N], BF, kind="ExternalInput")
    x2t = nc.dram_tensor("x2t", [D, N], BF, kind="ExternalInput")
    xgt_in = [x1t, x2t]
    xown_in = nc.dram_tensor("xown", [2, D, SH], BF, kind="ExternalInput")
    mtc_in = [nc.dram_tensor("mtc1", [NJ, 128, SH], FP8, kind="ExternalInput"),
              nc.dram_tensor("mtc2", [NJ, 128, SH], FP8, kind="ExternalInput")]
    wm_in = nc.dram_tensor("wm", [128, 15 * 128], BF, kind="ExternalInput")
    bs_in = nc.dram_tensor("bs", [128, 8], F32, kind="ExternalInput")
    out_ext = nc.dram_tensor("out", [2, SH, KU * 512], BF,
                             kind="ExternalOutput")

    # ---- internal DRAM for collectives ----
    rg = [list(range(NCORES))]
    ARD = FP8 if AR_FP8 else BF
    ar_in = [[nc.dram_tensor(f"ar_in_{l}_{g}", [2, 129, N // 2], ARD)
              for g in range(2)] for l in range(2)]
    ar_out = [[nc.dram_tensor(f"ar_out_{l}_{g}", [2, 129, N // 2], ARD,
                              addr_space="Shared")
               for g in range(2)] for l in range(2)]
    hag_in = [[nc.dram_tensor(f"hag_in_{l}_{g}", [D, SH], BF)
               for g in range(2)] for l in range(2)]
    hag_out = [[nc.dram_tensor(f"hag_out_{l}_{g}", [NCORES, D, SH], BF,
                               addr_space="Shared")
                for g in range(2)] for l in range(2)]
    wu_in = nc.dram_tensor("wu_in", [1, 64], BF)
    wu_out = nc.dram_tensor("wu_out", [1, 64], BF, addr_space="Shared")

    with tile.TileContext(nc) as tc:
        with (
            tc.tile_pool(name="const", bufs=1) as cpool,
            tc.tile_pool(name="mt", bufs=1) as mt_pool,
            tc.tile_pool(name="xt", bufs=1) as xt_pool,
            tc.tile_pool(name="xsel", bufs=1) as xsel_pool,
            tc.tile_pool(name="kq", bufs=2) as kq_pool,
            tc.tile_pool(name="es", bufs=4) as es_pool,
            tc.tile_pool(name="st", bufs=2) as st_pool,
            tc.tile_pool(name="tl", bufs=1) as tl_pool,
            tc.tile_pool(name="yu", bufs=1) as yu_pool,
            tc.tile_pool(name="sm", bufs=2) as sm_pool,
            tc.tile_pool(name="fz", bufs=4) as fz_pool,
            tc.tile_pool(name="psS", bufs=4, space="PSUM") as psS,
            tc.tile_pool(name="php", bufs=1, space="PSUM") as php_pool,
            tc.tile_pool(name="psC", bufs=2, space="PSUM") as psC,
        ):
            pid = nc.gpsimd.partition_id()
            pid_pe = nc.tensor.partition_id()

            # ---- constants (host-packed partition-major) ----
            wm = cpool.tile([128, 15 * 128], BF, name="wm_sb")
            nc.scalar.dma_start(wm[:], wm_in[:])
            bs = cpool.tile([128, 8], F32, name="bs_sb")
            nc.scalar.dma_start(bs[:], bs_in[:])
            rep_row = cpool.tile([1, 128], ARD, name="rep_row")
            nc.vector.memset(rep_row[:], REP_SCALE)
            ones8 = cpool.tile([128, 256], FP8, name="ones8")
            nc.vector.memset(ones8[:], CS_SCALE)
            # warm-up collective: synchronizes cores before the first real AR
            wu = cpool.tile([1, 64], BF, name="wu")
            nc.vector.memset(wu[:], 0.0)
            nc.scalar.dma_start(wu_in[:], wu[:])
            nc.gpsimd.collective_compute(
                "AllReduce", ADD, replica_groups=rg,
                ins=[wu_in[:]], outs=[wu_out[:]])

            def W(l, i):
                base = 7 * l + i if i < 7 else IDENT
                return wm[:, 128 * base:128 * (base + 1)]

            def B(l, i):
                return bs[:, 4 * l + i:4 * l + i + 1]

            ident = wm[:, 128 * IDENT:128 * (IDENT + 1)]

            # ---- initial loads (xown first: unblocks kv-proj) ----
            xown = []
            for g in range(2):
                t = sm_pool.tile([D, SH], BF, name=f"xown{g}_0", tag=f"xo{g}")
                nc.scalar.dma_start(t[:], xown_in[g])
                xown.append(t)
            xgt = []
            for g in range(2):
                t = xt_pool.tile([D, N], BF, name=f"x{g}t_0", tag=f"xt{g}")
                nc.scalar.dma_start(t[:], xgt_in[g][:])
                xgt.append(t)
            mt = []
            for g in range(2):
                t = mt_pool.tile([128, NJ * SH], FP8, name=f"mt{g}")
                nc.scalar.dma_start(
                    t.rearrange("p (j n) -> p j n", j=NJ),
                    mtc_in[g].ap().rearrange("j p n -> p j n"))
                mt.append(t)
            xsel = xsel_pool.tile([128, 12288], BF, name="xsel")

            hown_final = [None, None]
            state = {"xgt": xgt, "xown": xown}

            def proj_kv(l):
                xown = state["xown"]
                kt = kq_pool.tile([D, 2 * SH], BF, name=f"kt_{l}", tag="kt")
                vnat = kq_pool.tile([128, NT * 128], FP8, name=f"vn_{l}",
                                    tag="vn")
                for g in range(2):
                    psk = psS.tile([128, 512], F32, tag="psS",
                                   name=f"psk_{l}_{g}")
                    nc.tensor.matmul(psk[:], W(l, WK), xown[g][:],
                                     start=True, stop=True)
                    nc.vector.tensor_scalar(kt[:, g * SH:(g + 1) * SH],
                                            psk[:], B(l, BK), None, ADD)
                    psv = psS.tile([128, 512], F32, tag="psS",
                                   name=f"psv_{l}_{g}")
                    nc.tensor.matmul(psv[:], W(l, WV), xown[g][:],
                                     start=True, stop=True)
                    vt = st_pool.tile([128, SH], BF, name=f"vt_{l}_{g}",
                                      tag="vt")
                    nc.vector.tensor_copy(vt[:], psv[:])
                    pst = psC.tile([128, 512], BF, tag="psC",
                                   name=f"pst_{l}_{g}")
                    for j in range(4):
                        nc.tensor.transpose(pst[:, j * 128:(j + 1) * 128],
                                            vt[:, j * 128:(j + 1) * 128],
                                            ident)
                    # v scaled by V_SCALE (fp8/bf16 range headroom)
                    nc.vector.tensor_scalar(vnat[:, g * 512:(g + 1) * 512],
                                            pst[:], V_SCALE, None, MULT)
                return kt, vnat

            def proj_q(l, g):
                q = kq_pool.tile([D, N], BF, name=f"qt_{l}_{g}",
                                 tag=f"qt{g}", bufs=1)
                for c in range(8):
                    psq = psS.tile([128, 512], F32, tag="psS",
                                   name=f"psq_{l}_{g}_{c}")
                    nc.tensor.matmul(psq[:], W(l, WQ),
                                     state["xgt"][g][:, c * 512:(c + 1) * 512],
                                     start=True, stop=True)
                    if c % 2 == 0:
                        nc.vector.tensor_scalar(
                            q[:, c * 512:(c + 1) * 512], psq[:],
                            B(l, BQ), None, ADD)
                    else:
                        nc.scalar.activation(
                            q[:, c * 512:(c + 1) * 512], psq[:], IDN,
                            bias=B(l, BQ))
                return q

            def attention(l, g, kt, vnat, qt):
                for ic in range(NIC):
                    php = php_pool.tile([128, 1024], F32, tag="php",
                                        name=f"php_{l}_{g}_{ic}")
                    pcs = [psC.tile([128, 512], F32, tag="psC",
                                    name=f"pcs{h}_{l}_{g}_{ic}")
                           for h in range(2)]
                    for tt2 in range(NTP):
                        es = [es_pool.tile([128, 1024], FP8, tag="es",
                                           name=f"es{h}_{l}_{g}_{ic}_{tt2}")
                              for h in range(2)]
                        for j in range(2):
                            tt = 2 * tt2 + j
                            for h in range(2):
                                ps_s = psS.tile(
                                    [128, 512], F32, tag="psS",
                                    name=f"pss_{l}_{g}_{ic}_{tt}_{h}")
                                nc.tensor.matmul(
                                    ps_s[:], kt[:, tt * 128:(tt + 1) * 128],
                                    qt[:, ic * ICW + h * 512:
                                          ic * ICW + (h + 1) * 512],
                                    start=True, stop=True)
                                dst = es[h][:, j * 512:(j + 1) * 512]
                                if h == 0:
                                    nc.vector.tensor_scalar(
                                        dst.bitcast(U8), ps_s[:],
                                        ES_A, ES_B, MULT, ADD)
                                else:
                                    nc.scalar.activation(dst, ps_s[:], EXP,
                                                         scale=INV_SCALE)
                        for h in range(2):
                            esh = es[h].rearrange("p (k n) -> p k n", k=2)
                            nc.tensor.matmul(
                                php[:, h * 512:(h + 1) * 512],
                                vnat[:, tt2 * 256:(tt2 + 1) * 256]
                                .rearrange("p (k m) -> p k m", k=2),
                                esh, start=(tt2 == 0), stop=(tt2 == NTP - 1),
                                perf_mode=DR)
                            nc.tensor.matmul(
                                pcs[h][:],
                                ones8.rearrange("p (k m) -> p k m", k=2),
                                esh, start=(tt2 == 0), stop=(tt2 == NTP - 1),
                                perf_mode=DR)
                    pc = st_pool.tile([128, ICW], ARD, tag="pc")
                    cc = st_pool.tile([1, ICW], ARD, tag="cc")
                    hh, icq = divmod(ic, 2)
                    nc.vector.tensor_copy(pc[:, 0:512], php[:, 0:512])
                    nc.vector.tensor_copy(cc[:, 0:512], pcs[0][0:1, :])
                    nc.scalar.activation(pc[:, 512:1024], php[:, 512:1024],
                                         CPY)
                    nc.scalar.activation(cc[:, 512:1024], pcs[1][0:1, :],
                                         CPY)
                    for h in range(2):
                        sl = slice(icq * ICW + h * 512,
                                   icq * ICW + (h + 1) * 512)
                        nc.scalar.dma_start(ar_in[l][g][hh, 0:128, sl],
                                            pc[:, h * 512:(h + 1) * 512])
                        nc.scalar.dma_start(ar_in[l][g][hh, 128:129, sl],
                                            cc[:, h * 512:(h + 1) * 512])
                    if icq == 1:
                        nc.gpsimd.collective_compute(
                            "AllReduce", ADD, replica_groups=rg,
                            ins=[ar_in[l][g][hh]], outs=[ar_out[l][g][hh]])

            ls = [None, None]

            def xsage(l, g):
                xgt, xown = state["xgt"], state["xown"]
                yb = yu_pool.tile([128, N], FP8, name=f"yb_{l}_{g}", tag="yb")
                for jb in range(8):
                    psy = psS.tile([128, 512], F32, tag="psS",
                                   name=f"psy_{l}_{g}_{jb}")
                    for k in range(4):
                        jt = jb * 4 + k
                        nc.tensor.matmul(psy[:, k * 128:(k + 1) * 128],
                                         xgt[g][:, jt * 128:(jt + 1) * 128],
                                         W(l, WLS), start=True, stop=True)
                    if jb % 2 == 0:
                        nc.vector.tensor_copy(
                            yb[:, jb * 512:(jb + 1) * 512], psy[:])
                    else:
                        nc.scalar.activation(
                            yb[:, jb * 512:(jb + 1) * 512], psy[:], CPY)
                ps_a = psC.tile([128, 512], F32, tag="psC",
                                name=f"psa_{l}_{g}")
                for jp in range(NJP):
                    nc.tensor.matmul(
                        ps_a[:],
                        yb[:, jp * 256:(jp + 1) * 256]
                        .rearrange("p (k m) -> p k m", k=2),
                        mt[g][:, jp * 1024:(jp + 1) * 1024]
                        .rearrange("p (k n) -> p k n", k=2),
                        start=(jp == 0), stop=False, perf_mode=DR,
                        skip_group_check=True)
                nc.tensor.matmul(ps_a[:], W(l, WRS), xown[g][:],
                                 start=False, stop=True,
                                 skip_group_check=True)
                t = sm_pool.tile([128, SH], F32, name=f"ls_{l}_{g}",
                                 tag=f"ls{g}", bufs=1)
                nc.vector.tensor_scalar(t[:], ps_a[:], B(l, BL), None, ADD)
                ls[g] = t

            def tail(l, g):
                pfull = tl_pool.tile([128, N], ARD, name=f"pfull_{l}_{g}",
                                     tag="pfull")
                csrow = tl_pool.tile([1, N], ARD, name=f"csrow_{l}_{g}",
                                     tag="csrow")
                outt = tl_pool.tile([128, N], BF, name=f"outt_{l}_{g}",
                                    tag="outt")
                # hh0 loads on scalar (nothing queued behind); hh1 on
                # gpsimd so the scalar queue never head-of-line blocks on
                # the second AllReduce half
                nc.scalar.dma_start(pfull[:, 0:2048],
                                    ar_out[l][g][0, 0:128, :])
                nc.scalar.dma_start(csrow[:, 0:2048],
                                    ar_out[l][g][0, 128:129, :])
                nc.gpsimd.dma_start(pfull[:, 2048:4096],
                                    ar_out[l][g][1, 0:128, :])
                nc.gpsimd.dma_start(csrow[:, 2048:4096],
                                    ar_out[l][g][1, 128:129, :])
                for ch in range(8):
                    sl = slice(ch * 512, (ch + 1) * 512)
                    ps_rep = psS.tile([128, 512], F32, tag="psS",
                                      name=f"psrep_{l}_{g}_{ch}")
                    nc.tensor.matmul(ps_rep[:], rep_row[:], csrow[:, sl],
                                     start=True, stop=True)
                    rr = st_pool.tile([128, 512], F32, name=f"rr_{l}_{g}",
                                      tag="rr")
                    nc.vector.reciprocal_approx_fast(rr[:], ps_rep[:])
                    prod = st_pool.tile([128, 512], BF, name=f"prod_{l}_{g}",
                                        tag="prod")
                    nc.vector.tensor_tensor(prod[:], pfull[:, sl], rr[:],
                                            MULT)
                    nc.scalar.activation(outt[:, sl], prod[:], IDN,
                                         bias=B(l, BV))
                oown = st_pool.tile([128, 512], BF, name=f"oown_{l}_{g}",
                                    tag="oown")
                nc.gpsimd.dma_start(oown[:], outt[:, bass.ds(pid * SH, SH)])
                ub = yu_pool.tile([128, N], FP8, name=f"ub_{l}_{g}", tag="ub")
                for jb in range(8):
                    psu = psS.tile([128, 512], F32, tag="psS",
                                   name=f"psu_{l}_{g}_{jb}")
                    for k in range(4):
                        jt = jb * 4 + k
                        nc.tensor.matmul(psu[:, k * 128:(k + 1) * 128],
                                         outt[:, jt * 128:(jt + 1) * 128],
                                         W(l, WL1), start=True, stop=True)
                    if jb % 2 == 0:
                        nc.vector.tensor_copy(
                            ub[:, jb * 512:(jb + 1) * 512], psu[:])
                    else:
                        nc.scalar.activation(
                            ub[:, jb * 512:(jb + 1) * 512], psu[:], CPY)
                ps_a2 = psC.tile([128, 512], F32, tag="psC",
                                 name=f"psa2_{l}_{g}")
                for jp in range(NJP):
                    nc.tensor.matmul(
                        ps_a2[:],
                        ub[:, jp * 256:(jp + 1) * 256]
                        .rearrange("p (k m) -> p k m", k=2),
                        mt[g][:, jp * 1024:(jp + 1) * 1024]
                        .rearrange("p (k n) -> p k n", k=2),
                        start=(jp == 0), stop=False, perf_mode=DR,
                        skip_group_check=True)
                nc.tensor.matmul(ps_a2[:], W(l, WR1N), oown[:],
                                 start=False, stop=True,
                                 skip_group_check=True)
                h = sm_pool.tile([D, SH], BF, name=f"hown_{l}_{g}",
                                 tag=f"xo{g}")
                if l == 0:
                    t2 = st_pool.tile([128, 512], F32, name=f"t2_{l}_{g}",
                                      tag="t2")
                    nc.vector.tensor_tensor(t2[:], ls[g][:], ps_a2[:], SUB)
                    nc.vector.tensor_scalar(h[:], t2[:], 0.0, None, MAX)
                else:
                    nc.vector.tensor_tensor(h[:], ls[g][:], ps_a2[:], SUB)
                nc.scalar.dma_start(hag_in[l][g][:], h[:])
                nc.gpsimd.collective_compute(
                    "AllGather", mybir.AluOpType.bypass, replica_groups=rg,
                    ins=[hag_in[l][g][:]], outs=[hag_out[l][g][:]])
                if l == 0:
                    t = xt_pool.tile([D, N], BF, name=f"x{g}t_1",
                                     tag=f"xt{g}")
                    nc.gpsimd.dma_start(
                        t.rearrange("p (c n) -> p c n", c=NCORES),
                        hag_out[l][g].ap().rearrange("c p n -> p c n"))
                    state["xgt"][g] = t
                else:
                    nc.gpsimd.dma_start(
                        xsel[:, g * N:(g + 1) * N]
                        .rearrange("p (c n) -> p c n", c=NCORES),
                        hag_out[l][g].ap().rearrange("c p n -> p c n"))
                    if g == 0:
                        nc.gpsimd.dma_start(
                            xsel[:, 2 * N:3 * N]
                            .rearrange("p (c n) -> p c n", c=NCORES),
                            hag_out[l][g].ap().rearrange("c p n -> p c n"))
                return h

            # ================= layers =================
            for l in range(2):
                kt, vnat = proj_kv(l)
                qt0 = proj_q(l, 0)
                attention(l, 0, kt, vnat, qt0)
                qt1 = proj_q(l, 1)
                attention(l, 1, kt, vnat, qt1)
                xsage(l, 0)
                h0 = tail(l, 0)
                xsage(l, 1)
                h1 = tail(l, 1)
                state["xown"] = [h0, h1]
                if l == 1:
                    hown_final = [h0, h1]

            # ========== final adjacency (circulant symmetric) ==========
            def sig_chain(ps_z, z):
                sg1 = fz_pool.tile([128, 512], F32, tag="sg")
                nc.vector.tensor_scalar(sg1[:], ps_z[:], -SIG_A, SIG_LO,
                                        MULT, MAX)
                sg2 = fz_pool.tile([128, 512], F32, tag="sg")
                nc.vector.tensor_scalar(sg2[:].bitcast(I32), sg1[:],
                                        SIG_B, SIG_HI, ADD, MIN)
                sg3 = fz_pool.tile([128, 512], F32, tag="sg")
                nc.vector.tensor_scalar(sg3[:], sg2[:], 1.0, None, ADD)
                sg4 = fz_pool.tile([128, 512], F32, tag="sg")
                nc.vector.reciprocal_approx_fast(sg4[:], sg3[:])
                nc.vector.tensor_copy(z[:], sg4[:])

            def fin_unit(g, k, rhs, dve_rt):
                for rt in range(4):
                    ps_z = psS.tile([128, 512], F32, tag="psS",
                                    name=f"psz_{g}_{k}_{rt}")
                    nc.tensor.matmul(
                        ps_z[:],
                        hown_final[g][:, rt * 128:(rt + 1) * 128],
                        rhs, start=True, stop=True)
                    z = fz_pool.tile([128, 512], BF, tag="z")
                    if rt == dve_rt:
                        sig_chain(ps_z, z)
                    else:
                        nc.scalar.activation(z[:], ps_z[:], SIG)
                    nc.scalar.dma_start(
                        out_ext[g, rt * 128:(rt + 1) * 128,
                                k * 512:(k + 1) * 512],
                        z[:])

            # own-column units (k=0 and k=8) first: local, overlap last AG
            for g in range(2):
                fin_unit(g, 0, hown_final[g][:], 2)
            for g in range(2):
                fin_unit(g, 8, hown_final[1 - g][:], 2)
            for k in range(1, KU - 1):
                for g in range(2):
                    fin_unit(g, k,
                             xsel[:, bass.ds((pid_pe + 8 * g + k) * 512,
                                             512)],
                             2 if k % 2 == 0 else -1)

    nc.compile()
    return nc


def _host_prep(inputs):
    """Build per-core input maps from the full problem inputs."""
    x1 = np.asarray(inputs["x1"], np.float32)
    x2 = np.asarray(inputs["x2"], np.float32)
    x1t = np.ascontiguousarray(x1.T).astype(BF16)
    x2t = np.ascontiguousarray(x2.T).astype(BF16)

    def norm_adj_t(ei):
        ei = np.asarray(ei)
        A = np.zeros((N, N), np.float32)
        np.add.at(A, (ei[1], ei[0]), 1.0)
        deg = A.sum(1)
        A /= np.maximum(deg, 1.0)[:, None]
        return np.ascontiguousarray(A.T)  # MT[j, n]

    mt = [norm_adj_t(inputs["ei1"]), norm_adj_t(inputs["ei2"])]

    wm = np.zeros((15, 128, 128), np.float32)
    bs = np.zeros((8, 128, 1), np.float32)
    for l, s in enumerate(("1", "2")):
        wm[7 * l + WK] = inputs["Wk" + s]
        wm[7 * l + WQ] = inputs["Wq" + s]
        wm[7 * l + WV] = inputs["Wv" + s]
        wm[7 * l + WLS] = inputs["Wl" + s][:128] + inputs["Wl" + s][128:]
        wm[7 * l + WL1] = inputs["Wl" + s][128:]
        wm[7 * l + WRS] = inputs["Wr" + s][:128] + inputs["Wr" + s][128:]
        wm[7 * l + WR1N] = inputs["Wr" + s][128:]
        bs[4 * l + BK, :, 0] = inputs["bk" + s]
        bs[4 * l + BQ, :, 0] = inputs["bq" + s]
        bs[4 * l + BV, :, 0] = inputs["bv" + s]
        bs[4 * l + BL, :, 0] = inputs["bl" + s]
    wm[IDENT] = np.eye(128)
    wm_p = np.ascontiguousarray(
        wm.transpose(1, 0, 2).reshape(128, 15 * 128)).astype(BF16)
    bs_p = np.ascontiguousarray(bs[:, :, 0].T).astype(np.float32)

    in_maps = []
    for c in range(NCORES):
        sl = slice(c * SH, (c + 1) * SH)
        in_maps.append({
            "x1t": x1t,
            "x2t": x2t,
            "xown": np.stack([x1t[:, sl], x2t[:, sl]]),
            "mtc1": np.ascontiguousarray(
                mt[0][:, sl]).astype(E4M3).reshape(NJ, 128, SH),
            "mtc2": np.ascontiguousarray(
                mt[1][:, sl]).astype(E4M3).reshape(NJ, 128, SH),
            "wm": wm_p,
            "bs": bs_p,
        })
    return in_maps


def _assemble(results):
    """Place each core's circulant 9/16 blocks, mirror the rest."""
    full = np.empty((2 * N, 2 * N), np.float32)
    filled = np.zeros((16, 16), bool)
    for c in range(NCORES):
        o = np.asarray(results[c]["out"]).astype(np.float32)
        for gi in range(2):
            u = c + 8 * gi
            rows = slice(u * 512, (u + 1) * 512)
            for k in range(KU):
                cu = (u + k) % 16
                full[rows, cu * 512:(cu + 1) * 512] = \
                    o[gi][:, k * 512:(k + 1) * 512]
                filled[u, cu] = True
    for a in range(16):
        for b in range(16):
            if not filled[a, b]:
                full[a * 512:(a + 1) * 512, b * 512:(b + 1) * 512] = \
                    full[b * 512:(b + 1) * 512, a * 512:(a + 1) * 512].T
    return full


def get_nc():
    if "nc" not in _cache:
        _cache["nc"] = _build_nc()
    return _cache["nc"]


def kernel(**inputs):
    nc = get_nc()
    in_maps = _host_prep(inputs)
    res = run_bass_kernel_spmd(nc, in_maps, core_ids=list(range(NCORES)))
    return _assemble(res.results)


# revision 17
# speedup vs baseline: 1.1600x; 1.1600x over previous
"""Trainium2 Bass kernel for nn_AttMatch (2-graph attention + SAGEConv GNN).

Self-contained: takes the full unsharded inputs of the reference problem,
shards across 8 NeuronCores internally, runs one SPMD NEFF, and gathers the
full [8192, 8192] sigmoid adjacency output.

Implementation notes:
  * layer phases ordered for comms overlap: kv-proj -> q(0) -> attn(0) ->
    q(1) -> attn(1) -> xsage(0) -> tail(0) -> xsage(1) -> tail(1); q(1)
    emitted after attn(0) so the PE never head-of-line blocks on the
    previous layer's last AllGather.
  * one uniform [128,512] PSUM tag rotating over 4 banks: rotation depth 4
    breaks the exp->scores serialization chain that stalls the PE.
  * exp(scores) at [128,512] granularity: DVE Schraudolph (uint8 bit trick
    -> fp8e4m3) for query-half 0, scalar exact Exp->fp8 for half 1.
  * alpha^T@v and softmax column sums as fp8 DoubleRow matmuls (K=256 per
    instruction) accumulating in PSUM; M^T resident in SBUF as fp8 and both
    M-aggregations fp8 DoubleRow.
  * AllReduce per 2048-query half (bf16), launched mid-attention; a dummy
    warm-up AllReduce at kernel start absorbs cross-core skew.
  * final sigmoid(F@F^T) computes a balanced circulant 9/16 of the
    symmetric output (host mirrors the rest); the two own-column units
    (k=0,8) read local features and overlap the last AllGather; sigmoid
    splits between scalar and a DVE Schraudolph chain. Output bf16.
"""

import numpy as np
import ml_dtypes

import concourse.bass as bass
import concourse.bacc as bacc
import concourse.tile as tile
import concourse.mybir as mybir
from concourse.bass_utils import run_bass_kernel_spmd

BF16 = ml_dtypes.bfloat16
E4M3 = ml_dtypes.float8_e4m3

N = 4096          # nodes per graph
D = 128           # feature dim
NCORES = 8
SH = N // NCORES  # 512 nodes per graph per core
ICW = 1024        # query-chunk width
NIC = N // ICW    # 4 query chunks
NT = 2 * SH // 128   # 8 local target tiles
NTP = NT // 2     # 4 target-tile pairs (DoubleRow)
NJ = N // 128     # 32 source-node tiles
NJP = NJ // 2     # 16 source-tile pairs
KU = 9            # circulant column units in the final phase
INV_SCALE = 1.0 / np.sqrt(128.0)

V_SCALE = 0.25         # v pre-scale (headroom in bf16/fp8 paths)
CS_SCALE = 1.0 / 64.0  # ones value for column sums
REP_SCALE = V_SCALE / CS_SCALE  # =16: makes prod = P/colsum exactly
AR_FP8 = False  # AllReduce payload dtype (fp8 halves CC time; bf16 fallback)

LOG2E = 1.4426950408889634
ES_A = INV_SCALE * 8.0 * LOG2E
ES_B = 56.0 - 0.344 + 0.5
SIG_A = float(1 << 23) * LOG2E
SIG_B = 127.0 * float(1 << 23) - 360768.0
SIG_LO = -1.0e9
SIG_HI = 2.09e9

F32 = mybir.dt.float32
BF = mybir.dt.bfloat16
FP8 = mybir.dt.float8e4
U8 = mybir.dt.uint8
I32 = mybir.dt.int32
DR = mybir.MatmulPerfMode.DoubleRow

ADD = mybir.AluOpType.add
SUB = mybir.AluOpType.subtract
MULT = mybir.AluOpType.mult
MAX = mybir.AluOpType.max
MIN = mybir.AluOpType.min
EXP = mybir.ActivationFunctionType.Exp
IDN = mybir.ActivationFunctionType.Identity
CPY = mybir.ActivationFunctionType.Copy
SIG = mybir.ActivationFunctionType.Sigmoid

WK, WQ, WV, WLS, WL1, WRS, WR1N = range(7)
IDENT = 14
BK, BQ, BV, BL = range(4)

_cache = {}


def _build_nc():
    nc = bacc.Bacc("TRN2", target_bir_lowering=False, debug=False,
                   num_devices=NCORES)

    # ---- external I/O ----
    x1t = nc.dram_tensor("x1t", [D, N], BF, kind="ExternalInput")
    x2t = nc.dram_tensor("x2t", [D, N], BF, kind="ExternalInput")
    xgt_in = [x1t, x2t]
    xown_in = nc.dram_tensor("xown", [2, D, SH], BF, kind="ExternalInput")
    mtc_in = [nc.dram_tensor("mtc1", [NJ, 128, SH], FP8, kind="ExternalInput"),
              nc.dram_tensor("mtc2", [NJ, 128, SH], FP8, kind="ExternalInput")]
    wm_in = nc.dram_tensor("wm", [128, 15 * 128], BF, kind="ExternalInput")
    bs_in = nc.dram_tensor("bs", [128, 8], F32, kind="ExternalInput")
    out_ext = nc.dram_tensor("out", [2, SH, KU * 512], BF,
                             kind="ExternalOutput")

    # ---- internal DRAM for collectives ----
    rg = [list(range(NCORES))]
    ARD = FP8 if AR_FP8 else BF
    ar_in = [[nc.dram_tensor(f"ar_in_{l}_{g}", [2, 129, N // 2], ARD)
              for g in range(2)] for l in range(2)]
    ar_out = [[nc.dram_tensor(f"ar_out_{l}_{g}", [2, 129, N // 2], ARD,
                              addr_space="Shared")
               for g in range(2)] for l in range(2)]
    hag_in = [[nc.dram_tensor(f"hag_in_{l}_{g}", [D, SH], BF)
               for g in range(2)] for l in range(2)]
    hag_out = [[nc.dram_tensor(f"hag_out_{l}_{g}", [NCORES, D, SH], BF,
                               addr_space="Shared")
                for g in range(2)] for l in range(2)]
    wu_in = nc.dram_tensor("wu_in", [1, 64], BF)
    wu_out = nc.dram_tensor("wu_out", [1, 64], BF, addr_space="Shared")

    with tile.TileContext(nc) as tc:
        with (
            tc.tile_pool(name="const", bufs=1) as cpool,
            tc.tile_pool(name="mt", bufs=1) as mt_pool,
            tc.tile_pool(name="xt", bufs=1) as xt_pool,
            tc.tile_pool(name="xsel", bufs=1) as xsel_pool,
            tc.tile_pool(name="kq", bufs=2) as kq_pool,
            tc.tile_pool(name="es", bufs=4) as es_pool,
            tc.tile_pool(name="st", bufs=2) as st_pool,
            tc.tile_pool(name="tl", bufs=1) as tl_pool,
            tc.tile_pool(name="yu", bufs=1) as yu_pool,
            tc.tile_pool(name="sm", bufs=2) as sm_pool,
            tc.tile_pool(name="fz", bufs=4) as fz_pool,
            tc.tile_pool(name="psS", bufs=4, space="PSUM") as psS,
            tc.tile_pool(name="php", bufs=1, space="PSUM") as php_pool,
            tc.tile_pool(name="psC", bufs=2, space="PSUM") as psC,
        ):
            pid = nc.gpsimd.partition_id()
            pid_pe = nc.tensor.partition_id()

            # ---- constants (host-packed partition-major) ----
            wm = cpool.tile([128, 15 * 128], BF, name="wm_sb")
            nc.scalar.dma_start(wm[:], wm_in[:])
            bs = cpool.tile([128, 8], F32, name="bs_sb")
            nc.scalar.dma_start(bs[:], bs_in[:])
            rep_row = cpool.tile([1, 128], ARD, name="rep_row")
            nc.vector.memset(rep_row[:], REP_SCALE)
            ones8 = cpool.tile([128, 256], FP8, name="ones8")
            nc.vector.memset(ones8[:], CS_SCALE)
            # warm-up collective: synchronizes cores before the first real AR
            wu = cpool.tile([1, 64], BF, name="wu")
            nc.vector.memset(wu[:], 0.0)
            nc.scalar.dma_start(wu_in[:], wu[:])
            nc.gpsimd.collective_compute(
                "AllReduce", ADD, replica_groups=rg,
                ins=[wu_in[:]], outs=[wu_out[:]])

            def W(l, i):
                base = 7 * l + i if i < 7 else IDENT
                return wm[:, 128 * base:128 * (base + 1)]

            def B(l, i):
                return bs[:, 4 * l + i:4 * l + i + 1]

            ident = wm[:, 128 * IDENT:128 * (IDENT + 1)]

            # ---- initial loads (xown first: unblocks kv-proj) ----
            xown = []
            for g in range(2):
                t = sm_pool.tile([D, SH], BF, name=f"xown{g}_0", tag=f"xo{g}")
                nc.scalar.dma_start(t[:], xown_in[g])
                xown.append(t)
            xgt = []
            for g in range(2):
                t = xt_pool.tile([D, N], BF, name=f"x{g}t_0", tag=f"xt{g}")
                nc.scalar.dma_start(t[:], xgt_in[g][:])
                xgt.append(t)
            mt = []
            for g in range(2):
                t = mt_pool.tile([128, NJ * SH], FP8, name=f"mt{g}")
                nc.scalar.dma_start(
                    t.rearrange("p (j n) -> p j n", j=NJ),
                    mtc_in[g].ap().rearrange("j p n -> p j n"))
                mt.append(t)
            xsel = xsel_pool.tile([128, 12288], BF, name="xsel")

            hown_final = [None, None]
            state = {"xgt": xgt, "xown": xown}

            def proj_kv(l):
                xown = state["xown"]
                kt = kq_pool.tile([D, 2 * SH], BF, name=f"kt_{l}", tag="kt")
                vnat = kq_pool.tile([128, NT * 128], FP8, name=f"vn_{l}",
                                    tag="vn")
                for g in range(2):
                    psk = psS.tile([128, 512], F32, tag="psS",
                                   name=f"psk_{l}_{g}")
                    nc.tensor.matmul(psk[:], W(l, WK), xown[g][:],
                                     start=True, stop=True)
                    nc.vector.tensor_scalar(kt[:, g * SH:(g + 1) * SH],
                                            psk[:], B(l, BK), None, ADD)
                    psv = psS.tile([128, 512], F32, tag="psS",
                                   name=f"psv_{l}_{g}")
                    nc.tensor.matmul(psv[:], W(l, WV), xown[g][:],
                                     start=True, stop=True)
                    vt = st_pool.tile([128, SH], BF, name=f"vt_{l}_{g}",
                                      tag="vt")
                    nc.vector.tensor_copy(vt[:], psv[:])
                    pst = psC.tile([128, 512], BF, tag="psC",
                                   name=f"pst_{l}_{g}")
                    for j in range(4):
                        nc.tensor.transpose(pst[:, j * 128:(j + 1) * 128],
                                            vt[:, j * 128:(j + 1) * 128],
                                            ident)
                    # v scaled by V_SCALE (fp8/bf16 range headroom)
                    nc.vector.tensor_scalar(vnat[:, g * 512:(g + 1) * 512],
                                            pst[:], V_SCALE, None, MULT)
                return kt, vnat

            def proj_q(l, g):
                q = kq_pool.tile([D, N], BF, name=f"qt_{l}_{g}",
                                 tag=f"qt{g}", bufs=1)
                for c in range(8):
                    psq = psS.tile([128, 512], F32, tag="psS",
                                   name=f"psq_{l}_{g}_{c}")
                    nc.tensor.matmul(psq[:], W(l, WQ),
                                     state["xgt"][g][:, c * 512:(c + 1) * 512],
                                     start=True, stop=True)
                    if c % 2 == 0:
                        nc.vector.tensor_scalar(
                            q[:, c * 512:(c + 1) * 512], psq[:],
                            B(l, BQ), None, ADD)
                    else:
                        nc.scalar.activation(
                            q[:, c * 512:(c + 1) * 512], psq[:], IDN,
                            bias=B(l, BQ))
                return q

            def attention(l, g, kt, vnat, qt):
                for ic in range(NIC):
                    php = php_pool.tile([128, 1024], F32, tag="php",
                                        name=f"php_{l}_{g}_{ic}")
                    pcs = [psC.tile([128, 512], F32, tag="psC",
                                    name=f"pcs{h}_{l}_{g}_{ic}")
                           for h in range(2)]
                    for tt2 in range(NTP):
                        es = [es_pool.tile([128, 1024], FP8, tag="es",
                                           name=f"es{h}_{l}_{g}_{ic}_{tt2}")
                              for h in range(2)]
                        for j in range(2):
                            tt = 2 * tt2 + j
                            for h in range(2):
                                ps_s = psS.tile(
                                    [128, 512], F32, tag="psS",
                                    name=f"pss_{l}_{g}_{ic}_{tt}_{h}")
                                nc.tensor.matmul(
                                    ps_s[:], kt[:, tt * 128:(tt + 1) * 128],
                                    qt[:, ic * ICW + h * 512:
                                          ic * ICW + (h + 1) * 512],
                                    start=True, stop=True)
                                dst = es[h][:, j * 512:(j + 1) * 512]
                                if h == 0:
                                    nc.vector.tensor_scalar(
                                        dst.bitcast(U8), ps_s[:],
                                        ES_A, ES_B, MULT, ADD)
                                else:
                                    nc.scalar.activation(dst, ps_s[:], EXP,
                                                         scale=INV_SCALE)
                        for h in range(2):
                            esh = es[h].rearrange("p (k n) -> p k n", k=2)
                            nc.tensor.matmul(
                                php[:, h * 512:(h + 1) * 512],
                                vnat[:, tt2 * 256:(tt2 + 1) * 256]
                                .rearrange("p (k m) -> p k m", k=2),
                                esh, start=(tt2 == 0), stop=(tt2 == NTP - 1),
                                perf_mode=DR)
                            nc.tensor.matmul(
                                pcs[h][:],
                                ones8.rearrange("p (k m) -> p k m", k=2),
                                esh, start=(tt2 == 0), stop=(tt2 == NTP - 1),
                                perf_mode=DR)
                    pc = st_pool.tile([128, ICW], ARD, tag="pc")
                    cc = st_pool.tile([1, ICW], ARD, tag="cc")
                    hh, icq = divmod(ic, 2)
                    nc.vector.tensor_copy(pc[:, 0:512], php[:, 0:512])
                    nc.vector.tensor_copy(cc[:, 0:512], pcs[0][0:1, :])
                    nc.scalar.activation(pc[:, 512:1024], php[:, 512:1024],
                                         CPY)
                    nc.scalar.activation(cc[:, 512:1024], pcs[1][0:1, :],
                                         CPY)
                    for h in range(2):
                        sl = slice(icq * ICW + h * 512,
                                   icq * ICW + (h + 1) * 512)
                        nc.scalar.dma_start(ar_in[l][g][hh, 0:128, sl],
                                            pc[:, h * 512:(h + 1) * 512])
                        nc.scalar.dma_start(ar_in[l][g][hh, 128:129, sl],
                                            cc[:, h * 512:(h + 1) * 512])
                    if icq == 1:
                        nc.gpsimd.collective_compute(
                            "AllReduce", ADD, replica_groups=rg,
                            ins=[ar_in[l][g][hh]], outs=[ar_out[l][g][hh]])

            ls = [None, None]

            def xsage(l, g):
                xgt, xown = state["xgt"], state["xown"]
                yb = yu_pool.tile([128, N], FP8, name=f"yb_{l}_{g}", tag="yb")
                for jb in range(8):
                    psy = psS.tile([128, 512], F32, tag="psS",
                                   name=f"psy_{l}_{g}_{jb}")
                    for k in range(4):
                        jt = jb * 4 + k
                        nc.tensor.matmul(psy[:, k * 128:(k + 1) * 128],
                                         xgt[g][:, jt * 128:(jt + 1) * 128],
                                         W(l, WLS), start=True, stop=True)
                    if jb % 2 == 0:
                        nc.vector.tensor_copy(
                            yb[:, jb * 512:(jb + 1) * 512], psy[:])
                    else:
                        nc.scalar.activation(
                            yb[:, jb * 512:(jb + 1) * 512], psy[:], CPY)
                ps_a = psC.tile([128, 512], F32, tag="psC",
                                name=f"psa_{l}_{g}")
                for jp in range(NJP):
                    nc.tensor.matmul(
                        ps_a[:],
                        yb[:, jp * 256:(jp + 1) * 256]
                        .rearrange("p (k m) -> p k m", k=2),
                        mt[g][:, jp * 1024:(jp + 1) * 1024]
                        .rearrange("p (k n) -> p k n", k=2),
                        start=(jp == 0), stop=False, perf_mode=DR,
                        skip_group_check=True)
                nc.tensor.matmul(ps_a[:], W(l, WRS), xown[g][:],
                                 start=False, stop=True,
                                 skip_group_check=True)
                t = sm_pool.tile([128, SH], F32, name=f"ls_{l}_{g}",
                                 tag=f"ls{g}", bufs=1)
                nc.vector.tensor_scalar(t[:], ps_a[:], B(l, BL), None, ADD)
                ls[g] = t

            def tail(l, g):
                pfull = tl_pool.tile([128, N], ARD, name=f"pfull_{l}_{g}",
                                     tag="pfull")
                csrow = tl_pool.tile([1, N], ARD, name=f"csrow_{l}_{g}",
                                     tag="csrow")
                outt = tl_pool.tile([128, N], BF, name=f"outt_{l}_{g}",
                                    tag="outt")
                # hh0 loads on scalar (nothing queued behind); hh1 on
                # gpsimd so the scalar queue never head-of-line blocks on
                # the second AllReduce half
                nc.scalar.dma_start(pfull[:, 0:2048],
                                    ar_out[l][g][0, 0:128, :])
                nc.scalar.dma_start(csrow[:, 0:2048],
                                    ar_out[l][g][0, 128:129, :])
                nc.gpsimd.dma_start(pfull[:, 2048:4096],
                                    ar_out[l][g][1, 0:128, :])
                nc.gpsimd.dma_start(csrow[:, 2048:4096],
                                    ar_out[l][g][1, 128:129, :])
                ub = yu_pool.tile([128, N], FP8, name=f"ub_{l}_{g}", tag="ub")

                def epi(ch):
                    sl = slice(ch * 512, (ch + 1) * 512)
                    ps_rep = psS.tile([128, 512], F32, tag="psS",
                                      name=f"psrep_{l}_{g}_{ch}")
                    nc.tensor.matmul(ps_rep[:], rep_row[:], csrow[:, sl],
                                     start=True, stop=True)
                    rr = st_pool.tile([128, 512], F32, name=f"rr_{l}_{g}",
                                      tag="rr")
                    nc.vector.reciprocal_approx_fast(rr[:], ps_rep[:])
                    prod = st_pool.tile([128, 512], BF, name=f"prod_{l}_{g}",
                                        tag="prod")
                    nc.vector.tensor_tensor(prod[:], pfull[:, sl], rr[:],
                                            MULT)
                    nc.scalar.activation(outt[:, sl], prod[:], IDN,
                                         bias=B(l, BV))

                def ugen(jb):
                    psu = psS.tile([128, 512], F32, tag="psS",
                                   name=f"psu_{l}_{g}_{jb}")
                    for k in range(4):
                        jt = jb * 4 + k
                        nc.tensor.matmul(psu[:, k * 128:(k + 1) * 128],
                                         outt[:, jt * 128:(jt + 1) * 128],
                                         W(l, WL1), start=True, stop=True)
                    if jb % 2 == 0:
                        nc.vector.tensor_copy(
                            ub[:, jb * 512:(jb + 1) * 512], psu[:])
                    else:
                        nc.scalar.activation(
                            ub[:, jb * 512:(jb + 1) * 512], psu[:], CPY)

                # software-pipelined: epilogue+U-gen for the hh0 query half
                # run while the hh1 AllReduce half is still in flight
                for ch in range(4):
                    epi(ch)
                for jb in range(4):
                    ugen(jb)
                for ch in range(4, 8):
                    epi(ch)
                for jb in range(4, 8):
                    ugen(jb)
                oown = st_pool.tile([128, 512], BF, name=f"oown_{l}_{g}",
                                    tag="oown")
                nc.gpsimd.dma_start(oown[:], outt[:, bass.ds(pid * SH, SH)])
                ps_a2 = psC.tile([128, 512], F32, tag="psC",
                                 name=f"psa2_{l}_{g}")
                for jp in range(NJP):
                    nc.tensor.matmul(
                        ps_a2[:],
                        ub[:, jp * 256:(jp + 1) * 256]
                        .rearrange("p (k m) -> p k m", k=2),
                        mt[g][:, jp * 1024:(jp + 1) * 1024]
                        .rearrange("p (k n) -> p k n", k=2),
                        start=(jp == 0), stop=False, perf_mode=DR,
                        skip_group_check=True)
                nc.tensor.matmul(ps_a2[:], W(l, WR1N), oown[:],
                                 start=False, stop=True,
                                 skip_group_check=True)
                h = sm_pool.tile([D, SH], BF, name=f"hown_{l}_{g}",
                                 tag=f"xo{g}")
                if l == 0:
                    t2 = st_pool.tile([128, 512], F32, name=f"t2_{l}_{g}",
                                      tag="t2")
                    nc.vector.tensor_tensor(t2[:], ls[g][:], ps_a2[:], SUB)
                    nc.vector.tensor_scalar(h[:], t2[:], 0.0, None, MAX)
                else:
                    nc.vector.tensor_tensor(h[:], ls[g][:], ps_a2[:], SUB)
                nc.scalar.dma_start(hag_in[l][g][:], h[:])
                nc.gpsimd.collective_compute(
                    "AllGather", mybir.AluOpType.bypass, replica_groups=rg,
                    ins=[hag_in[l][g][:]], outs=[hag_out[l][g][:]])
                if l == 0:
                    t = xt_pool.tile([D, N], BF, name=f"x{g}t_1",
                                     tag=f"xt{g}")
                    nc.gpsimd.dma_start(
                        t.rearrange("p (c n) -> p c n", c=NCORES),
                        hag_out[l][g].ap().rearrange("c p n -> p c n"))
                    state["xgt"][g] = t
                else:
                    nc.gpsimd.dma_start(
                        xsel[:, g * N:(g + 1) * N]
                        .rearrange("p (c n) -> p c n", c=NCORES),
                        hag_out[l][g].ap().rearrange("c p n -> p c n"))
                    if g == 0:
                        nc.gpsimd.dma_start(
                            xsel[:, 2 * N:3 * N]
                            .rearrange("p (c n) -> p c n", c=NCORES),
                            hag_out[l][g].ap().rearrange("c p n -> p c n"))
                return h

            # ================= layers =================
            for l in range(2):
                kt, vnat = proj_kv(l)
                qt0 = proj_q(l, 0)
                attention(l, 0, kt, vnat, qt0)
                qt1 = proj_q(l, 1)
                attention(l, 1, kt, vnat, qt1)
                xsage(l, 0)
                h0 = tail(l, 0)
                xsage(l, 1)
                h1 = tail(l, 1)
                state["xown"] = [h0, h1]
                if l == 1:
                    hown_final = [h0, h1]

            # ========== final adjacency (circulant symmetric) ==========
            def sig_chain(ps_z, z):
                sg1 = fz_pool.tile([128, 512], F32, tag="sg")
                nc.vector.tensor_scalar(sg1[:], ps_z[:], -SIG_A, SIG_LO,
                                        MULT, MAX)
                sg2 = fz_pool.tile([128, 512], F32, tag="sg")
                nc.vector.tensor_scalar(sg2[:].bitcast(I32), sg1[:],
                                        SIG_B, SIG_HI, ADD, MIN)
                sg3 = fz_pool.tile([128, 512], F32, tag="sg")
                nc.vector.tensor_scalar(sg3[:], sg2[:], 1.0, None, ADD)
                sg4 = fz_pool.tile([128, 512], F32, tag="sg")
                nc.vector.reciprocal_approx_fast(sg4[:], sg3[:])
                nc.vector.tensor_copy(z[:], sg4[:])

            def fin_unit(g, k, rhs, dve_rt):
                for rt in range(4):
                    ps_z = psS.tile([128, 512], F32, tag="psS",
                                    name=f"psz_{g}_{k}_{rt}")
                    nc.tensor.matmul(
                        ps_z[:],
                        hown_final[g][:, rt * 128:(rt + 1) * 128],
                        rhs, start=True, stop=True)
                    z = fz_pool.tile([128, 512], BF, tag="z")
                    if rt == dve_rt:
                        sig_chain(ps_z, z)
                    else:
                        nc.scalar.activation(z[:], ps_z[:], SIG)
                    nc.scalar.dma_start(
                        out_ext[g, rt * 128:(rt + 1) * 128,
                                k * 512:(k + 1) * 512],
                        z[:])

            # own-column units (k=0 and k=8) first: local, overlap last AG
            for g in range(2):
                fin_unit(g, 0, hown_final[g][:], 2)
            for g in range(2):
                fin_unit(g, 8, hown_final[1 - g][:], 2)
            for k in range(1, KU - 1):
                for g in range(2):
                    fin_unit(g, k,
                             xsel[:, bass.ds((pid_pe + 8 * g + k) * 512,
                                             512)],
                             2 if k % 2 == 0 else -1)

    nc.compile()
    return nc


def _host_prep(inputs):
    """Build per-core input maps from the full problem inputs."""
    x1 = np.asarray(inputs["x1"], np.float32)
    x2 = np.asarray(inputs["x2"], np.float32)
    x1t = np.ascontiguousarray(x1.T).astype(BF16)
    x2t = np.ascontiguousarray(x2.T).astype(BF16)

    def norm_adj_t(ei):
        ei = np.asarray(ei)
        A = np.zeros((N, N), np.float32)
        np.add.at(A, (ei[1], ei[0]), 1.0)
        deg = A.sum(1)
        A /= np.maximum(deg, 1.0)[:, None]
        return np.ascontiguousarray(A.T)  # MT[j, n]

    mt = [norm_adj_t(inputs["ei1"]), norm_adj_t(inputs["ei2"])]

    wm = np.zeros((15, 128, 128), np.float32)
    bs = np.zeros((8, 128, 1), np.float32)
    for l, s in enumerate(("1", "2")):
        wm[7 * l + WK] = inputs["Wk" + s]
        wm[7 * l + WQ] = inputs["Wq" + s]
        wm[7 * l + WV] = inputs["Wv" + s]
        wm[7 * l + WLS] = inputs["Wl" + s][:128] + inputs["Wl" + s][128:]
        wm[7 * l + WL1] = inputs["Wl" + s][128:]
        wm[7 * l + WRS] = inputs["Wr" + s][:128] + inputs["Wr" + s][128:]
        wm[7 * l + WR1N] = inputs["Wr" + s][128:]
        bs[4 * l + BK, :, 0] = inputs["bk" + s]
        bs[4 * l + BQ, :, 0] = inputs["bq" + s]
        bs[4 * l + BV, :, 0] = inputs["bv" + s]
        bs[4 * l + BL, :, 0] = inputs["bl" + s]
    wm[IDENT] = np.eye(128)
    wm_p = np.ascontiguousarray(
        wm.transpose(1, 0, 2).reshape(128, 15 * 128)).astype(BF16)
    bs_p = np.ascontiguousarray(bs[:, :, 0].T).astype(np.float32)

    in_maps = []
    for c in range(NCORES):
        sl = slice(c * SH, (c + 1) * SH)
        in_maps.append({
            "x1t": x1t,
            "x2t": x2t,
            "xown": np.stack([x1t[:, sl], x2t[:, sl]]),
            "mtc1": np.ascontiguousarray(
                mt[0][:, sl]).astype(E4M3).reshape(NJ, 128, SH),
            "mtc2": np.ascontiguousarray(
                mt[1][:, sl]).astype(E4M3).reshape(NJ, 128, SH),
            "wm": wm_p,
            "bs": bs_p,
        })
    return in_maps


def _assemble(results):
    """Place each core's circulant 9/16 blocks, mirror the rest."""
    full = np.empty((2 * N, 2 * N), np.float32)
    filled = np.zeros((16, 16), bool)
    for c in range(NCORES):
        o = np.asarray(results[c]["out"]).astype(np.float32)
        for gi in range(2):
            u = c + 8 * gi
            rows = slice(u * 512, (u + 1) * 512)
            for k in range(KU):
                cu = (u + k) % 16
                full[rows, cu * 512:(cu + 1) * 512] = \
                    o[gi][:, k * 512:(k + 1) * 512]
                filled[u, cu] = True
    for a in range(16):
        for b in range(16):
            if not filled[a, b]:
                full[a * 512:(a + 1) * 512, b * 512:(b + 1) * 512] = \
                    full[b * 512:(b + 1) * 512, a * 512:(a + 1) * 512].T
    return full


def get_nc():
    if "nc" not in _cache:
        _cache["nc"] = _build_nc()
    return _cache["nc"]


def kernel(**inputs):
    nc = get_nc()
    in_maps = _host_prep(inputs)
    res = run_bass_kernel_spmd(nc, in_maps, core_ids=list(range(NCORES)))
    return _assemble(res.results)
